# revision 1
# baseline (speedup 1.0000x reference)
"""CEDiceMetrics Trainium2 kernel (nn_CEDiceMetrics_69148973466078).

Computes dice/tp/psum/tsum for input [2,8,128,192,192] f32 logits and
target [2,1,128,192,192] int32 labels, sharded over 8 NeuronCores by
splitting the flattened voxel dim.

Per-core device algorithm (tiles of [128 partitions, FD] voxels):
  1. A custom fused DVE op (EMBED_MAX_ANT, declared below and registered
     into the ant custom-DVE table) embeds each channel id into the 3 low
     mantissa bits of the f32 logits and takes the pairwise max in a
     single pass: out = max(a ^ ((a&7) ^ idA), b ^ ((b&7) ^ idB))
     (= (x & ~7) | id on raw bits; XOR form avoids a NaN-pattern
     constant, which HW canonicalizes). All embedded values are distinct
     so the max is tie-free; the <=8-ulp perturbation can only flip
     argmax winners for near-exact ties (~1e-4 relative count error worst
     case, far inside tolerance).
  2. Three more plain f32 max ops fold the four pair-maxes -> m;
     pred = (m as int32) & 7 is the argmax index, exactly.
  3. comb = 8*tgt + pred, kept in bf16 (values 0..63 exact) so the
     per-class tp masks [comb == 9c] run in the DVE 4x perf mode.
  4. tp[c]: bf16 masks are column-summed by the otherwise-idle TensorE
     (ones-vector matmuls accumulating into PSUM; a one-hot lhsT column
     routes each batch to its own PSUM row).
  5. psum/tsum via cumulative counts on the Scalar engine: Sign(v-(c+.5))
     with fused accum_out gives S = N - 2*count(v <= c); class counts are
     differences of adjacent cumulative counts (host side). Chunked so
     the tail threshold work overlaps compute.
Host glue sums the tiny per-core/per-partition partial counts and
evaluates the dice formula. Measured ~156 us on HW vs ~118 us HBM
roofline (42.5 MB/core @ ~358 GB/s); VectorE-bound.
"""

import sys

for _p in ("/root/.axon_site/_ro/trn_rl_repo",):
    if _p not in sys.path:
        sys.path.insert(0, _p)

import numpy as np
from contextlib import ExitStack

import concourse.bacc as bacc
import concourse.mybir as mybir
import concourse.tile as tile
from concourse.bass_utils import run_bass_kernel_spmd
import concourse.dve_ops as _dve_ops
from concourse.dve_ops import DveOp as _DveOp
from concourse.dve_spec import (AluOp as _AluOp, Bin as _Bin, Spec as _Spec,
                                Src0 as _Src0, Src1 as _Src1, C0 as _C0,
                                C1 as _C1, C2 as _C2)


def _embed_max_ref(in0, in1, s0, s1, imm2):
    # z = x ^ ((x & 7) ^ id)  ==  (x & ~7) | id on raw f32 bits
    sev = np.asarray(s0, np.float32).view(np.int32)
    ia = np.asarray(s1, np.float32).view(np.int32)
    ib = np.float32(imm2).view(np.int32)
    xa = np.asarray(in0, np.float32).view(np.int32)
    xb = np.asarray(in1, np.float32).view(np.int32)
    a = (xa ^ ((xa & sev) ^ ia)).view(np.float32)
    b = (xb ^ ((xb & sev) ^ ib)).view(np.float32)
    return np.maximum(a, b)


def _make_embed_max():
    import re as _re
    name = "EMBED_MAX_ANT"
    body = _Bin(
        _AluOp.MAX,
        _Bin(_AluOp.BITWISE_XOR, _Src0,
             _Bin(_AluOp.BITWISE_XOR, _Bin(_AluOp.BITWISE_AND, _Src0, _C0),
                  _C1)),
        _Bin(_AluOp.BITWISE_XOR, _Src1,
             _Bin(_AluOp.BITWISE_XOR, _Bin(_AluOp.BITWISE_AND, _Src1, _C0),
                  _C2)),
    )
    spec = _Spec(body=body, reference=_embed_max_ref)
    for op in _dve_ops.OPS:
        if op.name == name:
            return op
    if name not in _dve_ops._SUB_OPCODE_FOR_NAME:
        _dve_ops._SUB_OPCODE_FOR_NAME[name] = (
            max(_dve_ops._SUB_OPCODE_FOR_NAME.values()) + 1)
    probe = _DveOp(name, spec, subdim=False, uops_sha={})
    shas = {}
    try:
        probe.compile("v3")
    except ValueError as e:
        shas["v3"] = _re.search(r"v3: (\w+)", str(e)).group(1)
    op = _DveOp(name, spec, subdim=False, uops_sha=shas)
    _dve_ops.OPS.append(op)
    _dve_ops.CUSTOM_DVE_SPECS[name] = spec
    return op


_EMBED_MAX = _make_embed_max()


def _id_bits_float(i):
    return float(np.int32(i).view(np.float32))

# Problem geometry (hardcoded per spec).
B, C = 2, 8
D, H, W = 128, 192, 192
N = D * H * W                 # 4,718,592 voxels per (b, c)
NCORES = 8
NV = N // NCORES              # 589,824 voxels per core per batch
P = 128
FDC = NV // P                 # 4,608 free elems per partition per batch
FD = 1536                     # free elems per round
RPB = FDC // FD               # 3 rounds per batch
R = B * RPB                   # 6 rounds per core
EPS = 1e-5

_CACHE = {}


MM_N = 512                      # PSUM bank width in f32; matmul chunk
MM_CHUNKS = FDC // MM_N         # matmul chunks per batch-level mask


def _build_nc(with_bin0=False):
    nc = bacc.Bacc("TRN2", target_bir_lowering=False, debug=False,
                   num_devices=NCORES)
    x_dram = nc.dram_tensor("x", [B * C * P, FDC], mybir.dt.float32,
                            kind="ExternalInput")
    t_dram = nc.dram_tensor("tgt", [B * P, FDC], mybir.dt.int32,
                            kind="ExternalInput")
    tp_dram = nc.dram_tensor("tp_o", [B, C], mybir.dt.float32,
                             kind="ExternalOutput")
    ps_dram = nc.dram_tensor("ps_o", [P, 2 * B * (C - 1)], mybir.dt.float32,
                             kind="ExternalOutput")
    ts_dram = nc.dram_tensor("ts_o", [P, B * (C - 1)], mybir.dt.float32,
                             kind="ExternalOutput")

    xr = x_dram.ap().rearrange("(b c p) j -> b p c j", b=B, c=C)
    tr = t_dram.ap().rearrange("(b p) j -> b p j", b=B)

    with tile.TileContext(nc) as tc, ExitStack() as ctx:
        xpool = ctx.enter_context(tc.tile_pool(name="x", bufs=4))
        tpool = ctx.enter_context(tc.tile_pool(name="t", bufs=2))
        spool = ctx.enter_context(tc.tile_pool(name="s", bufs=2))
        mpool = ctx.enter_context(tc.tile_pool(name="m", bufs=2))
        apool = ctx.enter_context(tc.tile_pool(name="acc", bufs=1))
        ppool = ctx.enter_context(tc.tile_pool(name="ps", bufs=1,
                                               space="PSUM"))

        ps_cols = apool.tile([P, 2 * B * (C - 1)], mybir.dt.float32)
        ts_cols = apool.tile([P, B * (C - 1)], mybir.dt.float32)
        bias_t = apool.tile([P, C - 1], mybir.dt.float32)
        for i in range(C - 1):
            nc.vector.memset(bias_t[:, i:i + 1], -(i + 0.5))
        emb_c = apool.tile([P, 5], mybir.dt.int32)
        nc.vector.memset(emb_c[:, 0:1], 7)          # low-bit mask
        for q in range(4):
            nc.vector.memset(emb_c[:, q + 1:q + 2], 2 * q)   # idA per pair
        emb_cf = emb_c[:].bitcast(mybir.dt.float32)
        # one-hot lhsT per batch: ones in column b route the column-sums of
        # each mask chunk into PSUM row b.
        onehot = apool.tile([P, B * B], mybir.dt.bfloat16)
        for b in range(B):
            for j in range(B):
                nc.vector.memset(onehot[:, b * B + j:b * B + j + 1],
                                 1.0 if b == j else 0.0)
        lhsT = [onehot[:, b * B:(b + 1) * B] for b in range(B)]

        # per-class PSUM accumulators [B, MM_N]; class 0 only matters when
        # background is kept (it is sliced away for background=0)
        bins = list(range(0 if with_bin0 else 1, C))
        tp_psum = {c: ppool.tile([B, MM_N], mybir.dt.float32, tag=f"tp_ps{c}",
                                 name=f"tp_ps{c}")
                   for c in bins}

        mx = mybir.AluOpType.max
        sg = mybir.ActivationFunctionType.Sign
        HC = C // 2             # channels per half-load

        SPL = 2 * FD            # psum/mask early-chunk boundary
        QC = 2                  # channels per x load

        def load_x(b, q, f0):
            xt = xpool.tile([P, QC * FD], mybir.dt.float32, tag="xt",
                            name=f"xt_{b}_{q}_{f0}")
            nc.sync.dma_start(xt[:].rearrange("p (c j) -> p c j", c=QC),
                              xr[b, :, q * QC:(q + 1) * QC, f0:f0 + FD])
            return xt

        def load_x1(b, c, f0):
            xt = xpool.tile([P, FD], mybir.dt.float32, tag="xt1",
                            name=f"xt1_{b}_{c}_{f0}", bufs=2)
            nc.sync.dma_start(xt[:].rearrange("p (c j) -> p c j", c=1),
                              xr[b, :, c:c + 1, f0:f0 + FD])
            return xt

        for b in range(B):
            # first x channel before the target DMA so compute starts early
            x00 = load_x1(b, 0, 0)
            x01 = load_x1(b, 1, 0)

            tg = tpool.tile([P, FDC], mybir.dt.int32, tag="tg")
            nc.sync.dma_start(tg[:], tr[b])
            # tsum thresholds as soon as the target lands
            act_dump = spool.tile([P, FDC], mybir.dt.bfloat16, tag="act_dump",
                                  bufs=1)
            for i in range(C - 1):
                nc.scalar.activation(
                    act_dump[:], tg[:], sg, bias=bias_t[:, i:i + 1],
                    scale=1.0,
                    accum_out=ts_cols[:, b * (C - 1) + i:b * (C - 1) + i + 1])

            # comb = 8*tgt + pred, built in bf16 (values 0..63 are exact) so
            # the tp-mask tensor_scalars below hit the DVE 4x perf mode.
            comb = spool.tile([P, FDC], mybir.dt.bfloat16, tag="comb")
            nc.vector.tensor_scalar(comb[:], tg[:], 8, None,
                                    mybir.AluOpType.mult)
            pred = spool.tile([P, FDC], mybir.dt.int32, tag="pred")
            pred_bf = spool.tile([P, FDC], mybir.dt.bfloat16, tag="pred_bf")

            def emit_psum_chunk(lo, hi, col):
                for i in range(C - 1):
                    nc.scalar.activation(
                        act_dump[:, lo:hi], pred[:, lo:hi], sg,
                        bias=bias_t[:, i:i + 1], scale=1.0,
                        accum_out=ps_cols[:, col * (C - 1) + i:
                                          col * (C - 1) + i + 1])

            def emit_mask_chunk(lo, hi, first, last):
                for c in bins:
                    mask = mpool.tile([P, FDC], mybir.dt.bfloat16, tag="mask",
                                      name=f"mask_{b}_{lo}_{c}")
                    nc.vector.tensor_scalar(mask[:, lo:hi], comb[:, lo:hi],
                                            float(9 * c), None,
                                            mybir.AluOpType.is_equal)
                    for k in range(lo // MM_N, hi // MM_N):
                        nc.tensor.matmul(
                            tp_psum[c][:], lhsT[b],
                            mask[:, k * MM_N:(k + 1) * MM_N],
                            start=(first and k == lo // MM_N),
                            stop=(last and k == hi // MM_N - 1))

            for r in range(RPB):
                f0 = r * FD
                m03 = spool.tile([P, FD], mybir.dt.float32, tag="m03")
                for q in range(C // QC):
                    if r == 0 and q == 0:
                        ch = [x00[:], x01[:]]
                    else:
                        xt = load_x(b, q, f0)
                        ch = [xt[:, c * FD:(c + 1) * FD] for c in range(QC)]
                    # fused embed+max folds this channel pair in one DVE op
                    dst = m03[:] if q == 0 else ch[0]
                    nc.vector._custom_dve(
                        _EMBED_MAX, out=dst, in0=ch[0], in1=ch[1],
                        s0=emb_cf[:, 0:1], s1=emb_cf[:, q + 1:q + 2],
                        imm2=_id_bits_float(2 * q + 1))
                    if q > 0:
                        nc.vector.tensor_tensor(m03[:], m03[:], ch[0], mx)

                pr = pred[:, f0:f0 + FD]
                nc.vector.tensor_scalar(pr, m03[:].bitcast(mybir.dt.int32),
                                        7, None, mybir.AluOpType.bitwise_and)
                nc.vector.tensor_copy(pred_bf[:, f0:f0 + FD], pr)
                nc.vector.tensor_tensor(comb[:, f0:f0 + FD],
                                        comb[:, f0:f0 + FD],
                                        pred_bf[:, f0:f0 + FD],
                                        mybir.AluOpType.add)

                if f0 + FD == SPL:
                    emit_psum_chunk(0, SPL, 2 * b)
                    emit_mask_chunk(0, SPL, first=(b == 0), last=False)

            emit_psum_chunk(SPL, FDC, 2 * b + 1)
            emit_mask_chunk(SPL, FDC, first=False, last=(b == B - 1))

        # drain tp PSUM accumulators: [B, MM_N] -> [B, 1] each
        tp_sb = apool.tile([B, C], mybir.dt.float32)
        nc.vector.memset(tp_sb[:], 0.0)
        for c in bins:
            nc.vector.tensor_reduce(tp_sb[:, c:c + 1], tp_psum[c][:],
                                    mybir.AxisListType.X, mybir.AluOpType.add)
        nc.sync.dma_start(tp_dram.ap(), tp_sb[:])
        nc.sync.dma_start(ps_dram.ap(), ps_cols[:])
        nc.sync.dma_start(ts_dram.ap(), ts_cols[:])

    nc.compile()
    return nc


def _get_nc(with_bin0=False):
    key = f"nc{int(with_bin0)}"
    if key not in _CACHE:
        _CACHE[key] = _build_nc(with_bin0)
    return _CACHE[key]


def _make_in_maps(input, target):
    x = np.asarray(input, dtype=np.float32).reshape(B, C, N)
    t = np.asarray(target, dtype=np.int32).reshape(B, N)
    in_maps = []
    for k in range(NCORES):
        sl = slice(k * NV, (k + 1) * NV)
        xk = np.ascontiguousarray(x[:, :, sl]).reshape(B * C * P, FDC)
        tk = np.ascontiguousarray(t[:, sl]).reshape(B * P, FDC)
        in_maps.append({"x": xk, "tgt": tk})
    return in_maps


def _postprocess(results, background):
    # Sum partials over cores and partitions (already per-batch columns).
    tp = np.zeros((B, C), np.float64)
    ps_cols = np.zeros((B, C - 1), np.float64)
    ts_cols = np.zeros((B, C - 1), np.float64)
    for res in results:
        tp += res["tp_o"].astype(np.float64)
        ps_cols += (res["ps_o"].astype(np.float64).sum(0)
                    .reshape(B, 2, C - 1).sum(1))
        ts_cols += res["ts_o"].astype(np.float64).sum(0).reshape(B, C - 1)

    psum = np.zeros((B, C), np.float64)
    tsum = np.zeros((B, C), np.float64)
    for b in range(B):
        for cum, out in ((ps_cols, psum), (ts_cols, tsum)):
            s = cum[b]                            # S_c = N - 2*count(v <= c)
            f = (N - s) / 2.0                     # count(v <= c), c = 0..6
            full = np.concatenate([[0.0], f, [float(N)]])
            out[b] = np.diff(full)

    sl = slice(None) if background else slice(1, None)
    tp = tp[:, sl].astype(np.float32)
    psum = psum[:, sl].astype(np.float32)
    tsum = tsum[:, sl].astype(np.float32)
    dice = (np.float32(2.0) * tp / (psum + tsum + np.float32(EPS)))
    return dice.astype(np.float32), tp, psum, tsum


def _run(input, target, background, trace=False, **spmd_kwargs):
    nc = _get_nc(with_bin0=bool(background))
    in_maps = _make_in_maps(input, target)
    res = run_bass_kernel_spmd(nc, in_maps, list(range(NCORES)), trace=trace,
                               **spmd_kwargs)
    return _postprocess(res.results, background), res


def kernel(input, target, background):
    out, _ = _run(input, target, int(np.asarray(background)))
    return out



# revision 2
# speedup vs baseline: 1.0559x; 1.0559x over previous
"""CEDiceMetrics Trainium2 kernel (nn_CEDiceMetrics_69148973466078).

Computes dice/tp/psum/tsum for input [2,8,128,192,192] f32 logits and
target [2,1,128,192,192] int32 labels, sharded over 8 NeuronCores by
splitting the flattened voxel dim.

Per-core algorithm, engine-balanced against the ~110us/core HBM floor
(42.5 MB @ ~390 GB/s):

  1. Argmax over C=8: a fused DVE op embeds the channel id into f32
     bits [15:13] (clearing [15:0] so the later f32->fp16 truncation is
     exact) and takes pairwise max; the 3 fold maxes then run in fp16
     at the DVE 2x rate.  The <=0.8% value perturbation flips argmax
     only for near-ties (~0.2% of voxels, ~0.4% worst count error —
     well inside the 2e-2 tolerance; validated offline in numpy).
  2. A second fused DVE op turns the final fold into
     comb = tgt + 8*pred in fp16 (values 0..63 exact):
     bits->int via AND/OR-magic-constant, scale, plus the int32 tgt
     stream; its fused accum_out yields sum(comb) per partition, giving
     the first pred moment for free (used to recover psum class 1).
  3. tp counts: per-class [comb == 9c] masks in fp16 (DVE 4x mode),
     column-summed by TensorE matmuls into PSUM (classes paired two per
     PSUM bank via one-hot lhsT routing).
  4. psum: cumulative counts [pred >= c] split across engines to
     balance: classes 2..3 as DVE [comb >= 8c-.5] masks + matmul,
     classes 4..7 as ScalarE Sign thresholds with accum_out, class 1
     from the moment equation.
  5. tsum: 7 ScalarE Sign thresholds on the int32 target with accum_out.
Host glue sums the tiny per-core partials and evaluates dice.
"""

import sys

for _p in ("/root/.axon_site/_ro/trn_rl_repo",):
    if _p not in sys.path:
        sys.path.insert(0, _p)

import numpy as np
from contextlib import ExitStack

import concourse.bacc as bacc
import concourse.mybir as mybir
import concourse.tile as tile
from concourse.bass_utils import run_bass_kernel_spmd
import concourse.dve_ops as _dve_ops
from concourse.dve_ops import DveOp as _DveOp
from concourse.dve_spec import (AluOp as _AluOp, Bin as _Bin, Spec as _Spec,
                                Src0 as _Src0, Src1 as _Src1, C0 as _C0,
                                C1 as _C1, C2 as _C2)


def _register_op(name, spec):
    import re as _re
    for op in _dve_ops.OPS:
        if op.name == name:
            return op
    if name not in _dve_ops._SUB_OPCODE_FOR_NAME:
        _dve_ops._SUB_OPCODE_FOR_NAME[name] = (
            max(_dve_ops._SUB_OPCODE_FOR_NAME.values()) + 1)
    probe = _DveOp(name, spec, subdim=False, uops_sha={})
    shas = {}
    try:
        probe.compile("v3")
    except ValueError as e:
        shas["v3"] = _re.search(r"v3: (\w+)", str(e)).group(1)
    op = _DveOp(name, spec, subdim=False, uops_sha=shas)
    _dve_ops.OPS.append(op)
    _dve_ops.CUSTOM_DVE_SPECS[name] = spec
    return op


def _embed_max_ref(in0, in1, s0, s1, imm2):
    # z = x ^ ((x & mask) ^ id)  ==  (x & ~mask) | id on raw f32 bits
    msk = np.asarray(s0, np.float32).view(np.int32)
    ia = np.asarray(s1, np.float32).view(np.int32)
    ib = np.float32(imm2).view(np.int32)
    xa = np.asarray(in0, np.float32).view(np.int32)
    xb = np.asarray(in1, np.float32).view(np.int32)
    a = (xa ^ ((xa & msk) ^ ia)).view(np.float32)
    b = (xb ^ ((xb & msk) ^ ib)).view(np.float32)
    return np.maximum(a, b)


def _make_embed_max():
    body = _Bin(
        _AluOp.MAX,
        _Bin(_AluOp.BITWISE_XOR, _Src0,
             _Bin(_AluOp.BITWISE_XOR, _Bin(_AluOp.BITWISE_AND, _Src0, _C0),
                  _C1)),
        _Bin(_AluOp.BITWISE_XOR, _Src1,
             _Bin(_AluOp.BITWISE_XOR, _Bin(_AluOp.BITWISE_AND, _Src1, _C0),
                  _C2)),
    )
    return _register_op("EMBED_MAX_ANT",
                        _Spec(body=body, reference=_embed_max_ref))


def _extract_comb_ref(in0, in1, s0, s1, imm2):
    # comb = ((((m & s0bits) | s1bits) - s1) * imm2) + tgt ; accum = sum(comb)
    msk = np.asarray(s0, np.float32).view(np.int32)
    mag = np.asarray(s1, np.float32)
    magb = mag.view(np.int32) if mag.shape else np.float32(s1).view(np.int32)
    m = np.asarray(in0, np.float32).view(np.int32)
    p = (((m & msk) | magb).view(np.float32) - np.float32(s1)) * np.float32(imm2)
    out = p + np.asarray(in1, np.float32)
    return out, out.sum(axis=-1, keepdims=True)


def _make_extract_comb():
    body = _Bin(
        _AluOp.ADD,
        _Bin(_AluOp.MULTIPLY,
             _Bin(_AluOp.SUBTRACT,
                  _Bin(_AluOp.BITWISE_OR,
                       _Bin(_AluOp.BITWISE_AND, _Src0, _C0),
                       _C1),
                  _C1),
             _C2),
        _Src1,
    )
    return _register_op(
        "EXTRACT_COMB_ANT",
        _Spec(body=body, accum=_AluOp.ADD, reference=_extract_comb_ref))


_EMBED_MAX = _make_embed_max()
_EXTRACT_COMB = _make_extract_comb()


def _id_bits_float(i):
    return float(np.int32(i).view(np.float32))

# Problem geometry (hardcoded per spec).
B, C = 2, 8
D, H, W = 128, 192, 192
N = D * H * W                 # 4,718,592 voxels per batch (full)
NCORES = 8
NV = N // NCORES              # 589,824 voxels per core per batch
P = 128
FDC = NV // P                 # 4,608 free elems per partition per batch
FD = 768                      # free elems per round
RPB = FDC // FD               # 6 rounds per batch
CH = 1536                     # mask/threshold chunk width
CPB = FDC // CH               # 3 chunks per batch
MM_N = 512                    # PSUM bank width in f32; matmul chunk
EPS = 1e-5

PSV = (2, 3)                  # psum cum classes via DVE masks + TensorE
PSS = (4, 5, 6, 7)            # psum cum classes via ScalarE Sign accum

_CACHE = {}


def _build_nc(with_bin0=False):
    nc = bacc.Bacc("TRN2", target_bir_lowering=False, debug=False,
                   num_devices=NCORES)
    f16 = mybir.dt.float16
    f32 = mybir.dt.float32
    x_dram = nc.dram_tensor("x", [B * C * P, FDC], f32, kind="ExternalInput")
    t_dram = nc.dram_tensor("tgt", [B * P, FDC], mybir.dt.int32,
                            kind="ExternalInput")

    bins = list(range(0 if with_bin0 else 1, C))
    cls = [("tp", c) for c in bins] + [("ps", c) for c in PSV]
    npair = (len(cls) + 1) // 2
    # last slot index of each pair (for matmul stop flags)
    lastslot = [min(2 * i + 1, len(cls) - 1) for i in range(npair)]

    ncol_ts = B * (C - 1) * CPB
    ncol_ps = B * len(PSS) * CPB
    ncol_mo = B * RPB
    ncol = ncol_ts + ncol_ps + ncol_mo

    tp_dram = nc.dram_tensor("tp_o", [4, npair], f32, kind="ExternalOutput")
    sc_dram = nc.dram_tensor("sc_o", [P, ncol], f32, kind="ExternalOutput")

    xr = x_dram.ap().rearrange("(b c p) j -> b p c j", b=B, c=C)
    tr = t_dram.ap().rearrange("(b p) j -> b p j", b=B)

    mx = mybir.AluOpType.max
    sg = mybir.ActivationFunctionType.Sign

    with tile.TileContext(nc) as tc, ExitStack() as ctx:
        xpool = ctx.enter_context(tc.tile_pool(name="x", bufs=8))
        tpool = ctx.enter_context(tc.tile_pool(name="t", bufs=2))
        cpool = ctx.enter_context(tc.tile_pool(name="c", bufs=2))
        mpool = ctx.enter_context(tc.tile_pool(name="m", bufs=2))
        kpool = ctx.enter_context(tc.tile_pool(name="k", bufs=6))
        apool = ctx.enter_context(tc.tile_pool(name="acc", bufs=1))
        ppool = ctx.enter_context(tc.tile_pool(name="ps", bufs=1,
                                               space="PSUM"))

        sc_cols = apool.tile([P, ncol], f32)
        dump = apool.tile([P, CH], mybir.dt.bfloat16)

        # bias columns: tsum thresholds -(c-.5) c=1..7, then psum -(8c-.5)
        bias_t = apool.tile([P, (C - 1) + len(PSS)], f32)
        for i in range(C - 1):
            nc.vector.memset(bias_t[:, i:i + 1], -(i + 1 - 0.5))
        for i, c in enumerate(PSS):
            nc.vector.memset(bias_t[:, C - 1 + i:C + i], -(8 * c - 0.5))

        # bit-pattern constants for the custom DVE ops
        emb_c = apool.tile([P, 6], mybir.dt.int32)
        nc.vector.memset(emb_c[:, 0:1], 0xFFFF)           # embed clear mask
        for q in range(4):
            nc.vector.memset(emb_c[:, q + 1:q + 2], (2 * q) << 13)
        nc.vector.memset(emb_c[:, 5:6], 7 << 13)          # extract id mask
        emb_cf = emb_c[:].bitcast(f32)

        # one-hot lhsT variants: col block 4v+j = (j==v); variant
        # v = 2*half + b routes a mask's column sums to PSUM row v.
        onehot = apool.tile([P, 16], f16)
        for v in range(4):
            for j in range(4):
                nc.vector.memset(onehot[:, 4 * v + j:4 * v + j + 1],
                                 1.0 if v == j else 0.0)

        acc = [ppool.tile([4, MM_N], f32, tag=f"acc{i}", name=f"acc{i}")
               for i in range(npair)]

        def emit_chunk(b, cc, comb, tg):
            lo, hi = cc * CH, (cc + 1) * CH
            # ScalarE: tsum Sign thresholds on int32 tgt chunk
            for i in range(C - 1):
                col = b * (C - 1) * CPB + i * CPB + cc
                nc.scalar.activation(
                    dump[:], tg[:, lo:hi], sg, bias=bias_t[:, i:i + 1],
                    scale=1.0, accum_out=sc_cols[:, col:col + 1])
            # ScalarE: psum cumulative Sign thresholds on comb chunk
            for i in range(len(PSS)):
                col = ncol_ts + b * len(PSS) * CPB + i * CPB + cc
                nc.scalar.activation(
                    dump[:], comb[:, lo:hi], sg,
                    bias=bias_t[:, C - 1 + i:C + i],
                    scale=1.0, accum_out=sc_cols[:, col:col + 1])
            # DVE masks + TensorE column sums for tp and PSV classes
            for slot, (kind, c) in enumerate(cls):
                mask = kpool.tile([P, CH], f16, tag="mask",
                                  name=f"mask_{b}_{cc}_{slot}")
                if kind == "tp":
                    nc.vector.tensor_scalar(mask[:], comb[:, lo:hi],
                                            float(9 * c), None,
                                            mybir.AluOpType.is_equal)
                else:
                    nc.vector.tensor_scalar(mask[:], comb[:, lo:hi],
                                            8.0 * c - 0.5, None,
                                            mybir.AluOpType.is_ge)
                pair, half = divmod(slot, 2)
                lhsT = onehot[:, 4 * (2 * half + b):4 * (2 * half + b) + 4]
                for k in range(CH // MM_N):
                    first = (b == 0 and cc == 0 and k == 0 and half == 0)
                    last = (b == B - 1 and cc == CPB - 1
                            and k == CH // MM_N - 1 and slot == lastslot[pair])
                    nc.tensor.matmul(acc[pair][:], lhsT,
                                     mask[:, k * MM_N:(k + 1) * MM_N],
                                     start=first, stop=last)

        for b in range(B):
            tg = tpool.tile([P, FDC], mybir.dt.int32, tag="tg",
                            name=f"tg{b}")
            comb = cpool.tile([P, FDC], f16, tag="comb", name=f"comb{b}")

            for r in range(RPB):
                f0 = r * FD
                xt = [None] * 4
                for q in range(4):
                    xt[q] = xpool.tile([P, 2 * FD], f32, tag="xt",
                                       name=f"xt_{b}_{r}_{q}")
                    nc.sync.dma_start(
                        xt[q][:].rearrange("p (c j) -> p c j", c=2),
                        xr[b, :, 2 * q:2 * q + 2, f0:f0 + FD])
                if r == 0:
                    # target chunks land while round 0 computes
                    for cc in range(CPB):
                        nc.sync.dma_start(tg[:, cc * CH:(cc + 1) * CH],
                                          tr[b][:, cc * CH:(cc + 1) * CH])

                m = [None] * 4
                for q in range(4):
                    m[q] = mpool.tile([P, FD], f16, tag=f"m{q}",
                                      name=f"m_{b}_{r}_{q}")
                    nc.vector._custom_dve(
                        _EMBED_MAX, out=m[q][:],
                        in0=xt[q][:, 0:FD], in1=xt[q][:, FD:2 * FD],
                        s0=emb_cf[:, 0:1], s1=emb_cf[:, q + 1:q + 2],
                        imm2=_id_bits_float((2 * q + 1) << 13))
                nc.vector.tensor_tensor(m[0][:], m[0][:], m[1][:], mx)
                nc.vector.tensor_tensor(m[2][:], m[2][:], m[3][:], mx)
                nc.vector.tensor_tensor(m[0][:], m[0][:], m[2][:], mx)
                mocol = ncol_ts + ncol_ps + b * RPB + r
                nc.vector._custom_dve(
                    _EXTRACT_COMB, out=comb[:, f0:f0 + FD], in0=m[0][:],
                    in1=tg[:, f0:f0 + FD], s0=emb_cf[:, 5:6],
                    s1=8388608.0, imm2=float(8.0 / 8192.0),
                    accum_out=sc_cols[:, mocol:mocol + 1])

                if r % 2 == 1:
                    emit_chunk(b, r // 2, comb, tg)

        # drain PSUM accumulators [4, MM_N] -> [4, 1] each
        tp_sb = apool.tile([4, npair], f32)
        for i in range(npair):
            nc.vector.tensor_reduce(tp_sb[:, i:i + 1], acc[i][:],
                                    mybir.AxisListType.X, mybir.AluOpType.add)
        nc.sync.dma_start(tp_dram.ap(), tp_sb[:])
        nc.sync.dma_start(sc_dram.ap(), sc_cols[:])

    nc.compile()
    return nc


def _get_nc(with_bin0=False):
    key = f"nc{int(with_bin0)}"
    if key not in _CACHE:
        _CACHE[key] = _build_nc(with_bin0)
    return _CACHE[key]


def _make_in_maps(input, target):
    x = np.asarray(input, dtype=np.float32).reshape(B, C, N)
    t = np.asarray(target, dtype=np.int32).reshape(B, N)
    in_maps = []
    for k in range(NCORES):
        sl = slice(k * NV, (k + 1) * NV)
        xk = np.ascontiguousarray(x[:, :, sl]).reshape(B * C * P, FDC)
        tk = np.ascontiguousarray(t[:, sl]).reshape(B * P, FDC)
        in_maps.append({"x": xk, "tgt": tk})
    return in_maps


def _postprocess(results, background):
    with_bin0 = bool(background)
    bins = list(range(0 if with_bin0 else 1, C))
    cls = [("tp", c) for c in bins] + [("ps", c) for c in PSV]
    npair = (len(cls) + 1) // 2
    ncol_ts = B * (C - 1) * CPB
    ncol_ps = B * len(PSS) * CPB

    # Sum partials over cores (all quantities are additive counts).
    tp_o = np.zeros((4, npair), np.float64)
    sc = None
    for res in results:
        tp_o += res["tp_o"].astype(np.float64)
        s = res["sc_o"].astype(np.float64).sum(0)     # sum over partitions
        sc = s if sc is None else sc + s

    tp = np.zeros((B, C), np.float64)
    psum = np.zeros((B, C), np.float64)
    tsum = np.zeros((B, C), np.float64)
    cnt_ge_p = np.zeros((B, C + 1), np.float64)       # count(pred >= c)

    for slot, (kind, c) in enumerate(cls):
        pair, half = divmod(slot, 2)
        for b in range(B):
            v = tp_o[2 * half + b, pair]
            if kind == "tp":
                tp[b, c] = v
            else:
                cnt_ge_p[b, c] = v

    for b in range(B):
        # tsum from Sign accums: S = count(ge) - count(lt) = 2*cnt_ge - N
        cnt_ge_t = np.zeros(C + 1, np.float64)
        for i in range(C - 1):
            cols = [b * (C - 1) * CPB + i * CPB + cc for cc in range(CPB)]
            cnt_ge_t[i + 1] = (N + sc[cols].sum()) / 2.0
        for c in range(1, C):
            tsum[b, c] = cnt_ge_t[c] - cnt_ge_t[c + 1]
        tsum[b, 0] = N - cnt_ge_t[1]
        sum_tgt = sum(c * tsum[b, c] for c in range(1, C))

        for i, c in enumerate(PSS):
            cols = [ncol_ts + b * len(PSS) * CPB + i * CPB + cc
                    for cc in range(CPB)]
            cnt_ge_p[b, c] = (N + sc[cols].sum()) / 2.0
        for c in range(2, C):
            psum[b, c] = cnt_ge_p[b, c] - cnt_ge_p[b, c + 1]
        mo_cols = [ncol_ts + ncol_ps + b * RPB + r for r in range(RPB)]
        sum_comb = sc[mo_cols].sum()
        m1_pred = (sum_comb - sum_tgt) / 8.0
        psum[b, 1] = m1_pred - sum(c * psum[b, c] for c in range(2, C))
        psum[b, 0] = N - psum[b, 1:].sum()

    sl = slice(None) if background else slice(1, None)
    tp = tp[:, sl].astype(np.float32)
    psum = psum[:, sl].astype(np.float32)
    tsum = tsum[:, sl].astype(np.float32)
    dice = (np.float32(2.0) * tp / (psum + tsum + np.float32(EPS)))
    return dice.astype(np.float32), tp, psum, tsum


def _run(input, target, background, trace=False, **spmd_kwargs):
    nc = _get_nc(with_bin0=bool(background))
    in_maps = _make_in_maps(input, target)
    res = run_bass_kernel_spmd(nc, in_maps, list(range(NCORES)), trace=trace,
                               **spmd_kwargs)
    return _postprocess(res.results, background), res


def kernel(input, target, background):
    out, _ = _run(input, target, int(np.asarray(background)))
    return out


# revision 7
# speedup vs baseline: 1.0660x; 1.0096x over previous
"""CEDiceMetrics Trainium2 kernel (nn_CEDiceMetrics_69148973466078).

Computes dice/tp/psum/tsum for input [2,8,128,192,192] f32 logits and
target [2,1,128,192,192] int32 labels, sharded over 8 NeuronCores by
splitting the flattened voxel dim.

Per-core algorithm, engine-balanced against the ~110us/core HBM floor
(42.5 MB @ ~390 GB/s):

  1. Argmax over C=8: a fused DVE op embeds the channel id into f32
     bits [15:13] (clearing [15:0] so the later f32->fp16 truncation is
     exact) and takes pairwise max; the 3 fold maxes then run in fp16
     at the DVE 2x rate.  The <=0.8% value perturbation flips argmax
     only for near-ties (~0.2% of voxels, ~0.4% worst count error —
     well inside the 2e-2 tolerance; validated offline in numpy).
  2. A second fused DVE op turns the final fold into
     comb = tgt + 8*pred in bf16 (values 0..63 exact):
     bits->int via AND/OR-magic-constant, scale, plus the int32 tgt
     stream; its fused accum_out yields sum(comb) per partition, i.e.
     the first pred moment for free (recovers psum class 1).
  3. tp counts + psum cumulative classes 2..4: per-class bf16 masks
     (DVE 4x mode), column-summed by TensorE matmuls into PSUM
     (classes paired two per PSUM bank via one-hot lhsT routing);
     PSUM banks are DMA'd raw to the host, which does the final sum.
  4. psum cumulative classes 5..7: ScalarE Sign thresholds on comb
     with accum_out, chunk-aligned with comb production.
  5. tsum: 7 full-width ScalarE Sign thresholds on the int32 target
     (start as soon as the target lands; never on the critical tail).
Rounds shrink toward the batch end (1536,1536,1152,384) so the serial
tail after the last HBM load is only a few us. Host glue sums the tiny
per-core partials and evaluates dice.
"""

import sys

for _p in ("/root/.axon_site/_ro/trn_rl_repo",):
    if _p not in sys.path:
        sys.path.insert(0, _p)

import numpy as np
from contextlib import ExitStack

import concourse.bacc as bacc
import concourse.mybir as mybir
import concourse.tile as tile
from concourse.bass_utils import run_bass_kernel_spmd
import concourse.dve_ops as _dve_ops
from concourse.dve_ops import DveOp as _DveOp
from concourse.dve_spec import (AluOp as _AluOp, Bin as _Bin, Spec as _Spec,
                                Src0 as _Src0, Src1 as _Src1, C0 as _C0,
                                C1 as _C1, C2 as _C2)


def _register_op(name, spec):
    import re as _re
    for op in _dve_ops.OPS:
        if op.name == name:
            return op
    if name not in _dve_ops._SUB_OPCODE_FOR_NAME:
        _dve_ops._SUB_OPCODE_FOR_NAME[name] = (
            max(_dve_ops._SUB_OPCODE_FOR_NAME.values()) + 1)
    probe = _DveOp(name, spec, subdim=False, uops_sha={})
    shas = {}
    try:
        probe.compile("v3")
    except ValueError as e:
        shas["v3"] = _re.search(r"v3: (\w+)", str(e)).group(1)
    op = _DveOp(name, spec, subdim=False, uops_sha=shas)
    _dve_ops.OPS.append(op)
    _dve_ops.CUSTOM_DVE_SPECS[name] = spec
    return op


def _embed_max_ref(in0, in1, s0, s1, imm2):
    # z = x ^ ((x & mask) ^ id)  ==  (x & ~mask) | id on raw f32 bits
    msk = np.asarray(s0, np.float32).view(np.int32)
    ia = np.asarray(s1, np.float32).view(np.int32)
    ib = np.float32(imm2).view(np.int32)
    xa = np.asarray(in0, np.float32).view(np.int32)
    xb = np.asarray(in1, np.float32).view(np.int32)
    a = (xa ^ ((xa & msk) ^ ia)).view(np.float32)
    b = (xb ^ ((xb & msk) ^ ib)).view(np.float32)
    return np.maximum(a, b)


def _make_embed_max():
    body = _Bin(
        _AluOp.MAX,
        _Bin(_AluOp.BITWISE_XOR, _Src0,
             _Bin(_AluOp.BITWISE_XOR, _Bin(_AluOp.BITWISE_AND, _Src0, _C0),
                  _C1)),
        _Bin(_AluOp.BITWISE_XOR, _Src1,
             _Bin(_AluOp.BITWISE_XOR, _Bin(_AluOp.BITWISE_AND, _Src1, _C0),
                  _C2)),
    )
    return _register_op("EMBED_MAX_ANT",
                        _Spec(body=body, reference=_embed_max_ref))


def _extract_comb_ref(in0, in1, s0, s1, imm2):
    # comb = ((((m & s0bits) | s1bits) - s1) * imm2) + tgt ; accum = sum(comb)
    msk = np.asarray(s0, np.float32).view(np.int32)
    mag = np.asarray(s1, np.float32)
    magb = mag.view(np.int32) if mag.shape else np.float32(s1).view(np.int32)
    m = np.asarray(in0, np.float32).view(np.int32)
    p = (((m & msk) | magb).view(np.float32) - np.float32(s1)) * np.float32(imm2)
    out = p + np.asarray(in1, np.float32)
    return out, out.sum(axis=-1, keepdims=True)


def _make_extract_comb():
    body = _Bin(
        _AluOp.ADD,
        _Bin(_AluOp.MULTIPLY,
             _Bin(_AluOp.SUBTRACT,
                  _Bin(_AluOp.BITWISE_OR,
                       _Bin(_AluOp.BITWISE_AND, _Src0, _C0),
                       _C1),
                  _C1),
             _C2),
        _Src1,
    )
    return _register_op(
        "EXTRACT_COMB_ANT",
        _Spec(body=body, accum=_AluOp.ADD, reference=_extract_comb_ref))


_EMBED_MAX = _make_embed_max()
_EXTRACT_COMB = _make_extract_comb()


def _id_bits_float(i):
    return float(np.int32(i).view(np.float32))

# Problem geometry (hardcoded per spec).
B, C = 2, 8
D, H, W = 128, 192, 192
N = D * H * W                 # 4,718,592 voxels per batch (full)
NCORES = 8
NV = N // NCORES              # 589,824 voxels per core per batch
P = 128
FDC = NV // P                 # 4,608 free elems per partition per batch
EPS = 1e-5

# Rounds shrink toward the end so the post-last-DMA tail is small.
ROUNDS = [(0, 1536), (1536, 3072), (3072, 4224), (4224, 4608)]
RPB = len(ROUNDS)
# Counting chunks (mask / Sign spans); chunk i is complete after round i+1.
CHUNKS = [(0, 2304), (2304, 4224), (4224, 4608)]
CPB = len(CHUNKS)
TGW = 1536                    # target DMA chunk width

PSV = (2, 3, 4)               # psum cum classes via DVE masks + TensorE
PSS = (5, 6, 7)               # psum cum classes via ScalarE Sign accum
MM_N = 512

_CACHE = {}


def _mm_splits(lo, hi):
    ks = list(range(lo, hi, MM_N))
    return [(k, min(k + MM_N, hi)) for k in ks]


def _build_nc(with_bin0=False):
    nc = bacc.Bacc("TRN2", target_bir_lowering=False, debug=False,
                   num_devices=NCORES)
    f16 = mybir.dt.float16
    bf16 = mybir.dt.bfloat16
    f32 = mybir.dt.float32
    x_dram = nc.dram_tensor("x", [B * C * P, FDC], f32, kind="ExternalInput")
    t_dram = nc.dram_tensor("tgt", [B * P, FDC], mybir.dt.int32,
                            kind="ExternalInput")

    bins = list(range(0 if with_bin0 else 1, C))
    cls = [("tp", c) for c in bins] + [("ps", c) for c in PSV]
    npair = (len(cls) + 1) // 2
    lastslot = [min(2 * i + 1, len(cls) - 1) for i in range(npair)]

    ncol_ts = B * (C - 1)
    ncol_ps = B * len(PSS) * CPB
    ncol_mo = B * RPB

    tp_dram = nc.dram_tensor("tp_o", [4, npair * MM_N], f32,
                             kind="ExternalOutput")
    sc_dram = nc.dram_tensor("sc_o", [P, ncol_ts + ncol_ps], f32,
                             kind="ExternalOutput")
    mo_dram = nc.dram_tensor("mo_o", [P, ncol_mo], f32, kind="ExternalOutput")

    xr = x_dram.ap().rearrange("(b c p) j -> b p c j", b=B, c=C)
    tr = t_dram.ap().rearrange("(b p) j -> b p j", b=B)

    mx = mybir.AluOpType.max
    sg = mybir.ActivationFunctionType.Sign

    with tile.TileContext(nc) as tc, ExitStack() as ctx:
        xpool = ctx.enter_context(tc.tile_pool(name="x", bufs=6))
        tpool = ctx.enter_context(tc.tile_pool(name="t", bufs=2))
        cpool = ctx.enter_context(tc.tile_pool(name="c", bufs=2))
        mpool = ctx.enter_context(tc.tile_pool(name="m", bufs=2))
        kpool = ctx.enter_context(tc.tile_pool(name="k", bufs=3))
        apool = ctx.enter_context(tc.tile_pool(name="acc", bufs=1))
        ppool = ctx.enter_context(tc.tile_pool(name="ps", bufs=1,
                                               space="PSUM"))

        sc_cols = apool.tile([P, ncol_ts + ncol_ps], f32)
        mo_cols = apool.tile([P, ncol_mo], f32)
        dump = apool.tile([P, FDC], bf16)

        # bias columns: tsum thresholds -(c-.5) c=1..7, then psum -(8c-.5)
        bias_t = apool.tile([P, (C - 1) + len(PSS)], f32)
        for i in range(C - 1):
            nc.vector.memset(bias_t[:, i:i + 1], -(i + 1 - 0.5))
        for i, c in enumerate(PSS):
            nc.vector.memset(bias_t[:, C - 1 + i:C + i], -(8 * c - 0.5))

        # bit-pattern constants for the custom DVE ops
        emb_c = apool.tile([P, 6], mybir.dt.int32)
        nc.vector.memset(emb_c[:, 0:1], 0xFFFF)           # embed clear mask
        for q in range(4):
            nc.vector.memset(emb_c[:, q + 1:q + 2], (2 * q) << 13)
        nc.vector.memset(emb_c[:, 5:6], 7 << 13)          # extract id mask
        emb_cf = emb_c[:].bitcast(f32)

        # one-hot lhsT variants: col block 4v+j = (j==v); variant
        # v = 2*half + b routes a mask's column sums to PSUM row v.
        onehot = apool.tile([P, 16], bf16)
        for v in range(4):
            for j in range(4):
                nc.vector.memset(onehot[:, 4 * v + j:4 * v + j + 1],
                                 1.0 if v == j else 0.0)

        acc = [ppool.tile([4, MM_N], f32, tag=f"acc{i}", name=f"acc{i}")
               for i in range(npair)]

        def emit_chunk(b, cc, comb):
            lo, hi = CHUNKS[cc]
            # ScalarE: psum cumulative Sign thresholds on comb chunk
            for i in range(len(PSS)):
                col = ncol_ts + b * len(PSS) * CPB + i * CPB + cc
                nc.scalar.activation(
                    dump[:, lo:hi], comb[:, lo:hi], sg,
                    bias=bias_t[:, C - 1 + i:C + i],
                    scale=1.0, accum_out=sc_cols[:, col:col + 1])
            # DVE masks + TensorE column sums for tp and PSV classes
            for slot, (kind, c) in enumerate(cls):
                mask = kpool.tile([P, hi - lo], bf16, tag=f"mask{cc}",
                                  name=f"mask_{b}_{cc}_{slot}")
                if kind == "tp":
                    nc.vector.tensor_scalar(mask[:], comb[:, lo:hi],
                                            float(9 * c), None,
                                            mybir.AluOpType.is_equal)
                else:
                    nc.vector.tensor_scalar(mask[:], comb[:, lo:hi],
                                            8.0 * c - 0.5, None,
                                            mybir.AluOpType.is_ge)
                pair, half = divmod(slot, 2)
                lhsT = onehot[:, 4 * (2 * half + b):4 * (2 * half + b) + 4]
                splits = _mm_splits(lo, hi)
                for k, (alo, ahi) in enumerate(splits):
                    first = (b == 0 and cc == 0 and k == 0 and half == 0)
                    last = (b == B - 1 and cc == CPB - 1
                            and k == len(splits) - 1 and slot == lastslot[pair])
                    nc.tensor.matmul(acc[pair][:, 0:ahi - alo], lhsT,
                                     mask[:, alo - lo:ahi - lo],
                                     start=first, stop=last)

        for b in range(B):
            tg = tpool.tile([P, FDC], mybir.dt.int32, tag="tg",
                            name=f"tg{b}")
            comb = cpool.tile([P, FDC], bf16, tag="comb", name=f"comb{b}")

            for r, (f0, f1) in enumerate(ROUNDS):
                fd = f1 - f0
                xt = [None] * 4
                for q in range(4):
                    # fixed-size slot (max round width) so the pool ring
                    # reuses buffers across rounds of different widths
                    xt[q] = xpool.tile([P, 2 * 1536], f32, tag="xt",
                                       name=f"xt_{b}_{r}_{q}")
                    nc.sync.dma_start(
                        xt[q][:, 0:2 * fd].rearrange("p (c j) -> p c j", c=2),
                        xr[b, :, 2 * q:2 * q + 2, f0:f1])
                if r == 0:
                    # target chunks land while round 0 computes
                    for tcc in range(FDC // TGW):
                        nc.sync.dma_start(
                            tg[:, tcc * TGW:(tcc + 1) * TGW],
                            tr[b][:, tcc * TGW:(tcc + 1) * TGW])

                m = [None] * 4
                for q in range(4):
                    m[q] = mpool.tile([P, 1536], f16, tag=f"m{q}",
                                      name=f"m_{b}_{r}_{q}")
                    nc.vector._custom_dve(
                        _EMBED_MAX, out=m[q][:, 0:fd],
                        in0=xt[q][:, 0:fd], in1=xt[q][:, fd:2 * fd],
                        s0=emb_cf[:, 0:1], s1=emb_cf[:, q + 1:q + 2],
                        imm2=_id_bits_float((2 * q + 1) << 13))
                nc.vector.tensor_tensor(m[0][:, 0:fd], m[0][:, 0:fd],
                                        m[1][:, 0:fd], mx)
                nc.vector.tensor_tensor(m[2][:, 0:fd], m[2][:, 0:fd],
                                        m[3][:, 0:fd], mx)
                nc.vector.tensor_tensor(m[0][:, 0:fd], m[0][:, 0:fd],
                                        m[2][:, 0:fd], mx)
                mocol = b * RPB + r
                nc.vector._custom_dve(
                    _EXTRACT_COMB, out=comb[:, f0:f1], in0=m[0][:, 0:fd],
                    in1=tg[:, f0:f1], s0=emb_cf[:, 5:6],
                    s1=8388608.0, imm2=float(8.0 / 8192.0),
                    accum_out=mo_cols[:, mocol:mocol + 1])

                if r == 1:
                    # full-width tsum thresholds; never on the tail path
                    for i in range(C - 1):
                        col = b * (C - 1) + i
                        nc.scalar.activation(
                            dump[:], tg[:], sg, bias=bias_t[:, i:i + 1],
                            scale=1.0, accum_out=sc_cols[:, col:col + 1])
                if r >= 1:
                    emit_chunk(b, r - 1, comb)

        # drain PSUM accumulators via the (idle-at-tail) Scalar engine
        tp_sb = apool.tile([4, npair * MM_N], f32)
        for i in range(npair):
            nc.scalar.copy(tp_sb[:, i * MM_N:(i + 1) * MM_N], acc[i][:])
        nc.sync.dma_start(tp_dram.ap(), tp_sb[:])
        nc.sync.dma_start(sc_dram.ap(), sc_cols[:])
        nc.sync.dma_start(mo_dram.ap(), mo_cols[:])

    nc.compile()
    return nc


def _get_nc(with_bin0=False):
    key = f"nc{int(with_bin0)}"
    if key not in _CACHE:
        _CACHE[key] = _build_nc(with_bin0)
    return _CACHE[key]


def _make_in_maps(input, target):
    x = np.asarray(input, dtype=np.float32).reshape(B, C, N)
    t = np.asarray(target, dtype=np.int32).reshape(B, N)
    in_maps = []
    for k in range(NCORES):
        sl = slice(k * NV, (k + 1) * NV)
        xk = np.ascontiguousarray(x[:, :, sl]).reshape(B * C * P, FDC)
        tk = np.ascontiguousarray(t[:, sl]).reshape(B * P, FDC)
        in_maps.append({"x": xk, "tgt": tk})
    return in_maps


def _postprocess(results, background):
    with_bin0 = bool(background)
    bins = list(range(0 if with_bin0 else 1, C))
    cls = [("tp", c) for c in bins] + [("ps", c) for c in PSV]
    npair = (len(cls) + 1) // 2
    ncol_ts = B * (C - 1)

    # Sum partials over cores (all quantities are additive counts).
    tp_o = np.zeros((4, npair * MM_N), np.float64)
    sc = None
    mo = 0.0
    for res in results:
        tp_o += res["tp_o"].astype(np.float64)
        s = res["sc_o"].astype(np.float64).sum(0)     # sum over partitions
        sc = s if sc is None else sc + s
        mo = mo + res["mo_o"].astype(np.float64).sum(0)

    tp_bank = tp_o.reshape(4, npair, MM_N).sum(2)     # [4, npair]

    tp = np.zeros((B, C), np.float64)
    psum = np.zeros((B, C), np.float64)
    tsum = np.zeros((B, C), np.float64)
    cnt_ge_p = np.zeros((B, C + 1), np.float64)       # count(pred >= c)

    for slot, (kind, c) in enumerate(cls):
        pair, half = divmod(slot, 2)
        for b in range(B):
            v = tp_bank[2 * half + b, pair]
            if kind == "tp":
                tp[b, c] = v
            else:
                cnt_ge_p[b, c] = v

    for b in range(B):
        # tsum from Sign accums: S = count(ge) - count(lt) = 2*cnt_ge - N
        cnt_ge_t = np.zeros(C + 1, np.float64)
        for i in range(C - 1):
            cnt_ge_t[i + 1] = (N + sc[b * (C - 1) + i]) / 2.0
        for c in range(1, C):
            tsum[b, c] = cnt_ge_t[c] - cnt_ge_t[c + 1]
        tsum[b, 0] = N - cnt_ge_t[1]
        sum_tgt = sum(c * tsum[b, c] for c in range(1, C))

        for i, c in enumerate(PSS):
            cols = [ncol_ts + b * len(PSS) * CPB + i * CPB + cc
                    for cc in range(CPB)]
            cnt_ge_p[b, c] = (N + sc[cols].sum()) / 2.0
        for c in range(2, C):
            psum[b, c] = cnt_ge_p[b, c] - cnt_ge_p[b, c + 1]
        sum_comb = sum(mo[b * RPB + r] for r in range(RPB))
        m1_pred = (sum_comb - sum_tgt) / 8.0
        psum[b, 1] = m1_pred - sum(c * psum[b, c] for c in range(2, C))
        psum[b, 0] = N - psum[b, 1:].sum()

    sl = slice(None) if background else slice(1, None)
    tp = tp[:, sl].astype(np.float32)
    psum = psum[:, sl].astype(np.float32)
    tsum = tsum[:, sl].astype(np.float32)
    dice = (np.float32(2.0) * tp / (psum + tsum + np.float32(EPS)))
    return dice.astype(np.float32), tp, psum, tsum


def _run(input, target, background, trace=False, **spmd_kwargs):
    nc = _get_nc(with_bin0=bool(background))
    in_maps = _make_in_maps(input, target)
    res = run_bass_kernel_spmd(nc, in_maps, list(range(NCORES)), trace=trace,
                               **spmd_kwargs)
    return _postprocess(res.results, background), res


def kernel(input, target, background):
    out, _ = _run(input, target, int(np.asarray(background)))
    return out
